# revision 14
# baseline (speedup 1.0000x reference)
import sys
if "/opt/trn_rl_repo" not in sys.path:
    sys.path.insert(0, "/opt/trn_rl_repo")
import numpy as np
import ml_dtypes

import concourse.bacc as bacc
import concourse.bass as bass
import concourse.mybir as mybir
import concourse.tile as tile
from concourse.masks import make_identity
from concourse.bass_utils import run_bass_kernel_spmd

# Problem constants (hardcoded; kernel.py must be self-contained)
N_CORES = 8
B, D, H, W = 32, 256, 32, 32
K = 1024
HW = H * W              # 1024 tokens per batch image
B_LOC = B // N_CORES    # 4 batches per core
N_LOC = B_LOC * HW      # 4096 tokens per core
N_TILES = N_LOC // 128  # 32 token tiles per core
DECAY = 0.99
EPS = 1e-5
COMMITMENT = 0.25
N_TOT = B * HW          # 32768 tokens total
LOSS_SCALE = (1.0 + COMMITMENT) / float(N_TOT * D)

F32 = mybir.dt.float32
BF16 = mybir.dt.bfloat16
U32 = mybir.dt.uint32
I32 = mybir.dt.int32

# cc payload layout (flat f32)
CC_ESUM0 = 0                      # esum_T d-chunk0 [128,1024]
CC_ESUM1 = 128 * 1024             # esum_T d-chunk1 [128,1024]
CC_CNT = 2 * 128 * 1024           # counts [1024] (k-order)
CC_SCL = CC_CNT + 1024            # scalars [128,2] (xsq_vec, ssum_vec)
CC_LEN = CC_SCL + 256

_CACHED = {}


def build():
    nc = bacc.Bacc("TRN2", target_bir_lowering=False, debug=False,
                   enable_asserts=False, num_devices=N_CORES)

    # ---- DRAM I/O ----
    xloc = nc.dram_tensor("xloc", [B_LOC, D, HW], F32, kind="ExternalInput")
    e2t_hi = nc.dram_tensor("e2t_hi", [D, K], BF16, kind="ExternalInput")
    e2t_mi = nc.dram_tensor("e2t_mi", [D, K], BF16, kind="ExternalInput")
    nesq3 = nc.dram_tensor("nesq3", [3, K], BF16, kind="ExternalInput")
    emb_g = nc.dram_tensor("emb_g", [K, D], F32, kind="ExternalInput")
    avgT_in = nc.dram_tensor("avgT_in", [D, K], F32, kind="ExternalInput")
    cs_in = nc.dram_tensor("cs_in", [K], F32, kind="ExternalInput")

    xq_out = nc.dram_tensor("xq_out", [N_LOC, D], F32, kind="ExternalOutput")
    codes_out = nc.dram_tensor("codes_out", [N_LOC], U32, kind="ExternalOutput")
    ncs_out = nc.dram_tensor("ncs_out", [K], F32, kind="ExternalOutput")
    neaT_out = nc.dram_tensor("neaT_out", [D, K], F32, kind="ExternalOutput")
    enormT_out = nc.dram_tensor("enormT_out", [D, K], F32, kind="ExternalOutput")
    loss_out = nc.dram_tensor("loss_out", [1], F32, kind="ExternalOutput")

    with tile.TileContext(nc) as tc:
        _body(nc, tc, xloc, e2t_hi, e2t_mi, nesq3, emb_g, avgT_in, cs_in,
              xq_out, codes_out, ncs_out, neaT_out, enormT_out, loss_out)
    nc.compile()
    return nc


def _body(nc, tc, xloc, e2t_hi, e2t_mi, nesq3, emb_g, avgT_in, cs_in,
          xq_out, codes_out, ncs_out, neaT_out, enormT_out, loss_out):
    from contextlib import ExitStack
    ctx = ExitStack()
    with ctx:
        const = ctx.enter_context(tc.tile_pool(name="const", bufs=1))
        xpool = ctx.enter_context(tc.tile_pool(name="xpool", bufs=2))
        work = ctx.enter_context(tc.tile_pool(name="work", bufs=3))
        small = ctx.enter_context(tc.tile_pool(name="small", bufs=4))
        acc = ctx.enter_context(tc.tile_pool(name="acc", bufs=1))
        p_dist = ctx.enter_context(tc.tile_pool(name="p_dist", bufs=2, space="PSUM"))
        p_esum = ctx.enter_context(tc.tile_pool(name="p_esum", bufs=1, space="PSUM"))
        p_misc = ctx.enter_context(tc.tile_pool(name="p_misc", bufs=1, space="PSUM"))
        dram = ctx.enter_context(tc.tile_pool(name="dram", bufs=1, space="DRAM"))

        # ---- constants ----
        e_hi = const.tile([128, 2 * K], BF16, tag="e_hi")   # [d0 | d1] chunks
        e_mi = const.tile([128, 2 * K], BF16, tag="e_mi")
        nc.sync.dma_start(out=e_hi[:, 0:K], in_=e2t_hi[0:128, :])
        nc.sync.dma_start(out=e_hi[:, K:2 * K], in_=e2t_hi[128:256, :])
        nc.sync.dma_start(out=e_mi[:, 0:K], in_=e2t_mi[0:128, :])
        nc.sync.dma_start(out=e_mi[:, K:2 * K], in_=e2t_mi[128:256, :])
        nesq = const.tile([3, K], BF16, tag="nesq")
        nc.sync.dma_start(out=nesq[:], in_=nesq3[:, :])
        ones3 = const.tile([3, 128], BF16, tag="ones3")
        nc.vector.memset(ones3[:], 1.0)
        ones_col = const.tile([128, 1], BF16, tag="ones_col")
        nc.vector.memset(ones_col[:], 1.0)
        ones_f1 = const.tile([128, 1], F32, tag="ones_f1")
        nc.vector.memset(ones_f1[:], 1.0)
        ones_row = const.tile([1, 128], F32, tag="ones_row")
        nc.vector.memset(ones_row[:], 1.0)
        ident = const.tile([128, 128], BF16, tag="ident")
        make_identity(nc, ident[:])
        iota_i = const.tile([128, K], I32, tag="iota_i")
        nc.gpsimd.iota(iota_i[:], pattern=[[1, K]], base=0, channel_multiplier=0)
        iota_f = const.tile([128, K], F32, tag="iota_f")
        nc.vector.tensor_copy(out=iota_f[:], in_=iota_i[:])

        # ---- accumulators ----
        es_ps = [p_esum.tile([128, K], F32, tag=f"esum{c}", name=f"esum{c}") for c in range(2)]
        cnt_ps = p_misc.tile([128, 8], F32, tag="cnt")
        nc.vector.memset(cnt_ps[:], 0.0)
        ssum_parts = acc.tile([128, N_TILES], F32, tag="ssum_parts")
        xsq_parts = acc.tile([128, 8], F32, tag="xsq_parts")
        sq_scratch = acc.tile([128, HW], F32, tag="sq_scratch")

        # per-batch bf16 splits
        xb_hi = [None, None]
        xb_mi = [None, None]

        prev = None  # deferred (onehot, xT_sbuf, tile_idx) for esum/counts

        for b in range(B_LOC):
            xb = [xpool.tile([128, HW], F32, tag=f"xb{c}", name=f"xb{c}") for c in range(2)]
            for c in range(2):
                nc.sync.dma_start(out=xb[c][:], in_=xloc[b, 128 * c:128 * (c + 1), :])
            resid = [xpool.tile([128, HW], F32, tag=f"res{c}", name=f"res{c}") for c in range(2)]
            xb_hi = [xpool.tile([128, HW], BF16, tag=f"xh{c}", name=f"xh{c}") for c in range(2)]
            xb_mi = [xpool.tile([128, HW], BF16, tag=f"xm{c}", name=f"xm{c}") for c in range(2)]
            for c in range(2):
                nc.scalar.activation(out=xb_hi[c][:], in_=xb[c][:],
                                     func=mybir.ActivationFunctionType.Copy)
                nc.vector.tensor_tensor(out=resid[c][:], in0=xb[c][:],
                                        in1=xb_hi[c][:], op=mybir.AluOpType.subtract)
                nc.scalar.activation(out=xb_mi[c][:], in_=resid[c][:],
                                     func=mybir.ActivationFunctionType.Copy)

            for t in range(8):
                ti = b * 8 + t
                ts_ = slice(128 * t, 128 * (t + 1))

                # ---- distances (2 halves of K) ----
                dps = [p_dist.tile([128, 512], F32, tag="dist", name="dist") for _ in range(2)]
                for h in range(2):
                    hs = slice(512 * h, 512 * (h + 1))
                    mms = []
                    for c in range(2):
                        co = K * c
                        hs_c = slice(co + 512 * h, co + 512 * (h + 1))
                        mms += [(xb_hi[c][:, ts_], e_hi[:, hs_c]),
                                (xb_hi[c][:, ts_], e_mi[:, hs_c]),
                                (xb_mi[c][:, ts_], e_hi[:, hs_c])]
                    for j, (lhsT, rhs) in enumerate(mms):
                        nc.tensor.matmul(out=dps[h][:], lhsT=lhsT, rhs=rhs,
                                         start=(j == 0), stop=False)
                    nc.tensor.matmul(out=dps[h][:], lhsT=ones3[:], rhs=nesq[:, hs],
                                     start=False, stop=True)

                # ---- xT transposes (bf16) into one PSUM bank ----
                xt_ps = p_misc.tile([128, 512], BF16, tag="xt")
                for j, src in enumerate((xb_hi[0], xb_hi[1], xb_mi[0], xb_mi[1])):
                    nc.tensor.transpose(out=xt_ps[:, 128 * j:128 * (j + 1)],
                                        in_=src[:, ts_], identity=ident[:])
                xt_sb = work.tile([128, 512], BF16, tag="xt_sb")
                nc.scalar.activation(out=xt_sb[:], in_=xt_ps[:],
                                     func=mybir.ActivationFunctionType.Copy)

                # ---- argmax via top-8 per half, then combine ----
                mx = small.tile([128, 16], F32, tag="mx")
                mi = small.tile([128, 16], U32, tag="mi")
                for h in range(2):
                    nc.vector.max(out=mx[:, 8 * h:8 * h + 8], in_=dps[h][:])
                    nc.vector.max_index(out=mi[:, 8 * h:8 * h + 8],
                                        in_max=mx[:, 8 * h:8 * h + 8],
                                        in_values=dps[h][:])
                mif = small.tile([128, 2], F32, tag="mif")
                nc.vector.tensor_copy(out=mif[:], in_=mi[:, 0:16:8])
                # s* = max(vA, vB); code = vA>=vB ? iA : iB+512
                va, vb = mx[:, 0:1], mx[:, 8:9]
                cmp = small.tile([128, 1], F32, tag="cmp")
                nc.vector.tensor_tensor(out=cmp[:], in0=va, in1=vb,
                                        op=mybir.AluOpType.is_ge)
                t1 = small.tile([128, 1], F32, tag="t1")
                nc.vector.tensor_scalar(out=t1[:], in0=mif[:, 1:2], scalar1=512.0,
                                        scalar2=None, op0=mybir.AluOpType.add)
                t2 = small.tile([128, 1], F32, tag="t2")
                nc.vector.tensor_tensor(out=t2[:], in0=mif[:, 0:1], in1=t1[:],
                                        op=mybir.AluOpType.subtract)
                nc.vector.tensor_tensor(out=t2[:], in0=cmp[:], in1=t2[:],
                                        op=mybir.AluOpType.mult)
                codes_f = small.tile([128, 1], F32, tag="codes_f")
                nc.vector.tensor_tensor(out=codes_f[:], in0=t1[:], in1=t2[:],
                                        op=mybir.AluOpType.add)
                codes_u = small.tile([128, 1], U32, tag="codes_u")
                nc.vector.tensor_copy(out=codes_u[:], in_=codes_f[:])
                nc.sync.dma_start(out=codes_out[128 * ti:128 * (ti + 1)],
                                  in_=codes_u[:, 0])

                # ---- one-hot (gpsimd) ----
                onehot = work.tile([128, K], BF16, tag="onehot")
                nc.vector.tensor_scalar(out=onehot[:], in0=iota_f[:],
                                        scalar1=codes_f[:], scalar2=None,
                                        op0=mybir.AluOpType.is_equal)

                # ---- gather x_q rows, store ----
                xq = small.tile([128, D], F32, tag="xq")
                nc.gpsimd.indirect_dma_start(
                    out=xq[:], out_offset=None, in_=emb_g[:, :],
                    in_offset=bass.IndirectOffsetOnAxis(ap=codes_u[:, :1], axis=0))
                nc.sync.dma_start(out=xq_out[128 * ti:128 * (ti + 1), :], in_=xq[:])
                ld = small.tile([128, D], F32, tag="ld")
                nc.vector.tensor_tensor(out=ld[:], in0=xq[:], in1=xt_sb[:, 0:256],
                                        op=mybir.AluOpType.subtract)
                nc.vector.tensor_tensor(out=ld[:], in0=ld[:], in1=xt_sb[:, 256:512],
                                        op=mybir.AluOpType.subtract)
                nc.vector.tensor_tensor(out=ld[:], in0=ld[:], in1=ld[:],
                                        op=mybir.AluOpType.mult)
                nc.vector.reduce_sum(out=ssum_parts[:, ti:ti + 1], in_=ld[:],
                                     axis=mybir.AxisListType.X)

                # ---- deferred esum/counts from previous tile ----
                if prev is not None:
                    _esum_counts(nc, prev, es_ps, cnt_ps, ones_col, first=(prev[2] == 0))
                prev = (onehot, xt_sb, ti)

        _esum_counts(nc, prev, es_ps, cnt_ps, ones_col, first=False)

        # ================= tail =================
        cc_in = dram.tile([CC_LEN], F32, tag="cc_in")
        cc_out = dram.tile([CC_LEN], F32, tag="cc_out")
        es_sb = [work.tile([128, K], F32, tag=f"es_sb{c}", name=f"es_sb{c}") for c in range(2)]
        for c in range(2):
            nc.scalar.activation(out=es_sb[c][:], in_=es_ps[c][:],
                                 func=mybir.ActivationFunctionType.Copy)
            nc.sync.dma_start(
                out=cc_in[(CC_ESUM0, CC_ESUM1)[c]:(CC_ESUM0, CC_ESUM1)[c] + 128 * K],
                in_=es_sb[c][:])
        # counts [128p,8f] -> k-order flat
        cnt_sb = small.tile([128, 8], F32, tag="cnt_sb")
        nc.scalar.activation(out=cnt_sb[:], in_=cnt_ps[:],
                             func=mybir.ActivationFunctionType.Copy)
        nc.sync.dma_start(
            out=cc_in[CC_CNT:CC_CNT + K].rearrange("(c p) -> p c", p=128),
            in_=cnt_sb[:])
        ssum_v = small.tile([128, 1], F32, tag="ssum_v")
        nc.vector.reduce_sum(out=ssum_v[:], in_=ssum_parts[:], axis=mybir.AxisListType.X)
        scl = small.tile([128, 2], F32, tag="scl")
        nc.vector.memset(scl[:], 0.0)
        nc.vector.tensor_copy(out=scl[:, 0:1], in_=ssum_v[:])
        nc.sync.dma_start(
            out=cc_in[CC_SCL:CC_SCL + 256].rearrange("(f p) -> p f", p=128),
            in_=scl[:])

        nc.gpsimd.collective_compute(
            "AllReduce", mybir.AluOpType.add,
            replica_groups=[list(range(N_CORES))],
            ins=[cc_in[:].opt()], outs=[cc_out[:].opt()])

        # ---- EMA math (all cores identically) ----
        esA = [work.tile([128, K], F32, tag=f"es_sb{c}", name=f"esA{c}") for c in range(2)]
        for c in range(2):
            nc.sync.dma_start(
                out=esA[c][:],
                in_=cc_out[(CC_ESUM0, CC_ESUM1)[c]:(CC_ESUM0, CC_ESUM1)[c] + 128 * K]
                .rearrange("(p k) -> p k", p=128))
        cnt_all = small.tile([128, 8], F32, tag="cnt_all")
        nc.sync.dma_start(
            out=cnt_all[:],
            in_=cc_out[CC_CNT:CC_CNT + K].rearrange("(c p) -> p c", p=128))
        scl_all = small.tile([128, 2], F32, tag="scl_all")
        nc.sync.dma_start(
            out=scl_all[:],
            in_=cc_out[CC_SCL:CC_SCL + 256].rearrange("(f p) -> p f", p=128))
        csz = small.tile([128, 8], F32, tag="csz")
        nc.sync.dma_start(out=csz[:], in_=cs_in[:].rearrange("(c p) -> p c", p=128))

        # ncs = DECAY*cs + (1-DECAY)*counts
        ncs = small.tile([128, 8], F32, tag="ncs")
        tA = small.tile([128, 8], F32, tag="tA")
        nc.vector.tensor_scalar(out=ncs[:], in0=csz[:], scalar1=DECAY,
                                scalar2=None, op0=mybir.AluOpType.mult)
        nc.vector.tensor_scalar(out=tA[:], in0=cnt_all[:], scalar1=1.0 - DECAY,
                                scalar2=None, op0=mybir.AluOpType.mult)
        nc.vector.tensor_tensor(out=ncs[:], in0=ncs[:], in1=tA[:],
                                op=mybir.AluOpType.add)
        nc.sync.dma_start(out=ncs_out[:].rearrange("(c p) -> p c", p=128), in_=ncs[:])

        # n = sum(ncs) -> broadcast [128,1]
        nrow = small.tile([128, 1], F32, tag="nrow")
        nc.vector.reduce_sum(out=nrow[:], in_=ncs[:], axis=mybir.AxisListType.X)
        n_ps = p_misc.tile([1, 1], F32, tag="xt")
        nc.tensor.matmul(out=n_ps[:], lhsT=nrow[:], rhs=ones_f1[:],
                         start=True, stop=True)
        n_sb = small.tile([1, 1], F32, tag="n_sb")
        nc.scalar.activation(out=n_sb[:], in_=n_ps[:],
                             func=mybir.ActivationFunctionType.Copy)
        nb_ps = p_misc.tile([128, 1], F32, tag="xt")
        nc.tensor.matmul(out=nb_ps[:], lhsT=ones_row[:], rhs=n_sb[:],
                         start=True, stop=True)
        nbc = small.tile([128, 1], F32, tag="nbc")
        nc.scalar.activation(out=nbc[:], in_=nb_ps[:],
                             func=mybir.ActivationFunctionType.Copy)

        # cs_k = (ncs+EPS) / (n + K*EPS) * n ; rec_cs = 1/cs_k
        denom = small.tile([128, 1], F32, tag="denom")
        nc.vector.tensor_scalar(out=denom[:], in0=nbc[:], scalar1=float(K) * EPS,
                                scalar2=None, op0=mybir.AluOpType.add)
        rden = small.tile([128, 1], F32, tag="rden")
        nc.vector.reciprocal(out=rden[:], in_=denom[:])
        fac = small.tile([128, 1], F32, tag="fac")
        nc.vector.tensor_tensor(out=fac[:], in0=nbc[:], in1=rden[:],
                                op=mybir.AluOpType.mult)
        cs_k = small.tile([128, 8], F32, tag="cs_k")
        nc.vector.tensor_scalar(out=cs_k[:], in0=ncs[:], scalar1=EPS,
                                scalar2=None, op0=mybir.AluOpType.add)
        nc.vector.tensor_scalar(out=cs_k[:], in0=cs_k[:], scalar1=fac[:],
                                scalar2=None, op0=mybir.AluOpType.mult)
        rcs = small.tile([128, 8], F32, tag="rcs")
        nc.vector.reciprocal(out=rcs[:], in_=cs_k[:])
        # roundtrip rec_cs to get row layout [1, 1024]
        rcs_d = dram.tile([K], F32, tag="rcs_d")
        nc.sync.dma_start(out=rcs_d[:].rearrange("(c p) -> p c", p=128), in_=rcs[:])
        rcs_row = small.tile([1, K], F32, tag="rcs_row")
        nc.sync.dma_start(out=rcs_row[:], in_=rcs_d[None, :])
        # broadcast to [128, 1024] via PE (2 halves)
        rb_ps = [p_dist.tile([128, 512], F32, tag="dist", name="rb_ps") for _ in range(2)]
        for h in range(2):
            nc.tensor.matmul(out=rb_ps[h][:], lhsT=ones_row[:],
                             rhs=rcs_row[:, 512 * h:512 * (h + 1)],
                             start=True, stop=True)

        # neaT = DECAY*avgT + (1-DECAY)*esum ; enormT = neaT * rec_cs
        avgT = [xpool.tile([128, K], F32, tag=f"xb{c}", name=f"avgT{c}") for c in range(2)]
        for c in range(2):
            nc.sync.dma_start(out=avgT[c][:], in_=avgT_in[128 * c:128 * (c + 1), :])
        for c in range(2):
            na = xpool.tile([128, K], F32, tag=f"res{c}")
            nc.scalar.activation(out=na[:], in_=esA[c][:],
                                 func=mybir.ActivationFunctionType.Copy,
                                 scale=1.0 - DECAY)
            nc.vector.tensor_scalar(out=avgT[c][:], in0=avgT[c][:], scalar1=DECAY,
                                    scalar2=None, op0=mybir.AluOpType.mult)
            nc.vector.tensor_tensor(out=na[:], in0=na[:], in1=avgT[c][:],
                                    op=mybir.AluOpType.add)
            nc.sync.dma_start(out=neaT_out[128 * c:128 * (c + 1), :], in_=na[:])
            en = xpool.tile([128, K], F32, tag=f"xh{c}")
            for h in range(2):
                nc.vector.tensor_tensor(out=en[:, 512 * h:512 * (h + 1)],
                                        in0=na[:, 512 * h:512 * (h + 1)],
                                        in1=rb_ps[h][:], op=mybir.AluOpType.mult)
            nc.sync.dma_start(out=enormT_out[128 * c:128 * (c + 1), :], in_=en[:])

        # loss = ((xsq - ssum) * LOSS_SCALE); xsq/ssum are partition-sums of scl_all
        sc_ps = p_misc.tile([2, 1], F32, tag="xt")
        nc.tensor.matmul(out=sc_ps[:], lhsT=scl_all[:], rhs=ones_f1[:],
                         start=True, stop=True)
        sc_sb = small.tile([2, 1], F32, tag="sc_sb")
        nc.scalar.activation(out=sc_sb[:], in_=sc_ps[:],
                             func=mybir.ActivationFunctionType.Copy)
        sc_d = dram.tile([2], F32, tag="sc_d")
        nc.sync.dma_start(out=sc_d[:], in_=sc_sb[:, 0])
        sc_row = small.tile([1, 2], F32, tag="sc_row")
        nc.sync.dma_start(out=sc_row[:], in_=sc_d[None, :])
        lt = small.tile([1, 1], F32, tag="lt")
        nc.vector.tensor_scalar(out=lt[:], in0=sc_row[:, 0:1], scalar1=LOSS_SCALE,
                                scalar2=None, op0=mybir.AluOpType.mult)
        nc.sync.dma_start(out=loss_out[:], in_=lt[:, 0])


def _esum_counts(nc, prev, es_ps, cnt_ps, ones_col, first):
    onehot, xt_sb, ti = prev
    st = (ti == 0)
    for c in range(2):
        for s in range(2):  # hi, mi
            lhsT = xt_sb[:, 256 * s + 128 * c:256 * s + 128 * (c + 1)]
            for h in range(2):
                nc.tensor.matmul(out=es_ps[c][:, 512 * h:512 * (h + 1)],
                                 lhsT=lhsT, rhs=onehot[:, 512 * h:512 * (h + 1)],
                                 start=(st and s == 0), stop=False)
    for kc in range(8):
        nc.tensor.matmul(out=cnt_ps[:, kc:kc + 1],
                         lhsT=onehot[:, 128 * kc:128 * (kc + 1)], rhs=ones_col[:],
                         start=False, stop=False)


def _split3_neg_esq(emb):
    esq = np.sum(emb.astype(np.float64) * emb.astype(np.float64), axis=1)
    v = (-esq).astype(np.float32)
    h = v.astype(ml_dtypes.bfloat16)
    r = v - h.astype(np.float32)
    m = r.astype(ml_dtypes.bfloat16)
    l = (r - m.astype(np.float32)).astype(ml_dtypes.bfloat16)
    return np.stack([h, m, l])


def _split2(v):
    h = v.astype(ml_dtypes.bfloat16)
    m = (v - h.astype(np.float32)).astype(ml_dtypes.bfloat16)
    return h, m


def kernel(x, emb, cluster_size, embed_avg, _trace=False):
    x = np.ascontiguousarray(np.asarray(x, dtype=np.float32))
    emb = np.ascontiguousarray(np.asarray(emb, dtype=np.float32))
    cluster_size = np.asarray(cluster_size, dtype=np.float32)
    embed_avg = np.asarray(embed_avg, dtype=np.float32)

    if "nc" not in _CACHED:
        _CACHED["nc"] = build()
    nc = _CACHED["nc"]

    e2t = np.ascontiguousarray((2.0 * emb).T)            # [D, K]
    e2t_hi, e2t_mi = _split2(e2t)
    nesq3 = _split3_neg_esq(emb)                          # [3, K]
    avgT = np.ascontiguousarray(embed_avg.T)              # [D, K]

    in_maps = []
    for c in range(N_CORES):
        xs = x[B_LOC * c:B_LOC * (c + 1)].reshape(B_LOC, D, HW)
        in_maps.append({
            "xloc": np.ascontiguousarray(xs),
            "e2t_hi": e2t_hi, "e2t_mi": e2t_mi, "nesq3": nesq3,
            "emb_g": emb, "avgT_in": avgT, "cs_in": cluster_size,
        })

    res = run_bass_kernel_spmd(nc, in_maps, core_ids=list(range(N_CORES)),
                               trace=_trace)
    outs = res.results
    _CACHED["outs"] = outs
    if _trace:
        _CACHED["last_result"] = res

    # ---- unshard ----
    xq_st = np.empty((B, D, H, W), np.float32)
    codes_map = np.empty((B, HW), np.int32)
    for c in range(N_CORES):
        o = outs[c]
        xq = o["xq_out"].reshape(B_LOC, H, W, D).transpose(0, 3, 1, 2)
        xq_st[B_LOC * c:B_LOC * (c + 1)] = xq
        codes_map[B_LOC * c:B_LOC * (c + 1)] = \
            o["codes_out"].view(np.int32).reshape(B_LOC, HW)
    codes_map = codes_map.reshape(B, H, W)
    o0 = outs[0]
    vq_loss = np.float32(o0["loss_out"][0])
    new_cluster_size = o0["ncs_out"]
    new_embed_avg = np.ascontiguousarray(o0["neaT_out"].T)
    embed_normalized = np.ascontiguousarray(o0["enormT_out"].T)
    return (xq_st, vq_loss, codes_map, new_cluster_size, new_embed_avg,
            embed_normalized)


# revision 15
# speedup vs baseline: 1.0252x; 1.0252x over previous
import sys
if "/opt/trn_rl_repo" not in sys.path:
    sys.path.insert(0, "/opt/trn_rl_repo")
import numpy as np
import ml_dtypes

import concourse.bacc as bacc
import concourse.bass as bass
import concourse.mybir as mybir
import concourse.tile as tile
from concourse.masks import make_identity
from concourse.bass_utils import run_bass_kernel_spmd

# Problem constants (hardcoded; kernel.py must be self-contained)
N_CORES = 8
B, D, H, W = 32, 256, 32, 32
K = 1024
HW = H * W              # 1024 tokens per batch image
B_LOC = B // N_CORES    # 4 batches per core
N_LOC = B_LOC * HW      # 4096 tokens per core
N_TILES = N_LOC // 128  # 32 token tiles per core
DECAY = 0.99
EPS = 1e-5
COMMITMENT = 0.25
N_TOT = B * HW          # 32768 tokens total
LOSS_SCALE = (1.0 + COMMITMENT) / float(N_TOT * D)

F32 = mybir.dt.float32
BF16 = mybir.dt.bfloat16
U32 = mybir.dt.uint32
I32 = mybir.dt.int32

# cc payload layout (flat f32)
CC_ESUM0 = 0                      # esum_T d-chunk0 [128,1024]
CC_ESUM1 = 128 * 1024             # esum_T d-chunk1 [128,1024]
CC_CNT = 2 * 128 * 1024           # counts [1024] (k-order)
CC_SCL = CC_CNT + 1024            # scalars [128,2] (xsq_vec, ssum_vec)
CC_LEN = CC_SCL + 256

_CACHED = {}


def build():
    nc = bacc.Bacc("TRN2", target_bir_lowering=False, debug=False,
                   enable_asserts=False, num_devices=N_CORES)

    # ---- DRAM I/O ----
    xloc = nc.dram_tensor("xloc", [B_LOC, D, HW], F32, kind="ExternalInput")
    e2t_hi = nc.dram_tensor("e2t_hi", [D, K], BF16, kind="ExternalInput")
    e2t_mi = nc.dram_tensor("e2t_mi", [D, K], BF16, kind="ExternalInput")
    nesq3 = nc.dram_tensor("nesq3", [3, K], BF16, kind="ExternalInput")
    emb_g = nc.dram_tensor("emb_g", [K, D], F32, kind="ExternalInput")
    avgT_in = nc.dram_tensor("avgT_in", [D, K], F32, kind="ExternalInput")
    cs_in = nc.dram_tensor("cs_in", [K], F32, kind="ExternalInput")

    xq_out = nc.dram_tensor("xq_out", [N_LOC, D], F32, kind="ExternalOutput")
    codes_out = nc.dram_tensor("codes_out", [N_LOC], U32, kind="ExternalOutput")
    ncs_out = nc.dram_tensor("ncs_out", [K], F32, kind="ExternalOutput")
    neaT_out = nc.dram_tensor("neaT_out", [D, K], F32, kind="ExternalOutput")
    enormT_out = nc.dram_tensor("enormT_out", [D, K], F32, kind="ExternalOutput")
    loss_out = nc.dram_tensor("loss_out", [1], F32, kind="ExternalOutput")

    with tile.TileContext(nc) as tc:
        _body(nc, tc, xloc, e2t_hi, e2t_mi, nesq3, emb_g, avgT_in, cs_in,
              xq_out, codes_out, ncs_out, neaT_out, enormT_out, loss_out)
    nc.compile()
    return nc


def _body(nc, tc, xloc, e2t_hi, e2t_mi, nesq3, emb_g, avgT_in, cs_in,
          xq_out, codes_out, ncs_out, neaT_out, enormT_out, loss_out):
    from contextlib import ExitStack
    ctx = ExitStack()
    with ctx:
        const = ctx.enter_context(tc.tile_pool(name="const", bufs=1))
        xpool = ctx.enter_context(tc.tile_pool(name="xpool", bufs=2))
        work = ctx.enter_context(tc.tile_pool(name="work", bufs=3))
        small = ctx.enter_context(tc.tile_pool(name="small", bufs=4))
        acc = ctx.enter_context(tc.tile_pool(name="acc", bufs=1))
        p_dist = ctx.enter_context(tc.tile_pool(name="p_dist", bufs=2, space="PSUM"))
        p_esum = ctx.enter_context(tc.tile_pool(name="p_esum", bufs=1, space="PSUM"))
        p_misc = ctx.enter_context(tc.tile_pool(name="p_misc", bufs=1, space="PSUM"))
        dram = ctx.enter_context(tc.tile_pool(name="dram", bufs=1, space="DRAM"))

        # ---- constants ----
        e_hi = const.tile([128, 2 * K], BF16, tag="e_hi")   # [d0 | d1] chunks
        e_mi = const.tile([128, 2 * K], BF16, tag="e_mi")
        nc.sync.dma_start(out=e_hi[:, 0:K], in_=e2t_hi[0:128, :])
        nc.sync.dma_start(out=e_hi[:, K:2 * K], in_=e2t_hi[128:256, :])
        nc.sync.dma_start(out=e_mi[:, 0:K], in_=e2t_mi[0:128, :])
        nc.sync.dma_start(out=e_mi[:, K:2 * K], in_=e2t_mi[128:256, :])
        nesq = const.tile([3, K], BF16, tag="nesq")
        nc.sync.dma_start(out=nesq[:], in_=nesq3[:, :])
        ones3 = const.tile([3, 128], BF16, tag="ones3")
        nc.vector.memset(ones3[:], 1.0)
        ones_col = const.tile([128, 1], BF16, tag="ones_col")
        nc.vector.memset(ones_col[:], 1.0)
        ones_f1 = const.tile([128, 1], F32, tag="ones_f1")
        nc.vector.memset(ones_f1[:], 1.0)
        ones_row = const.tile([1, 128], F32, tag="ones_row")
        nc.vector.memset(ones_row[:], 1.0)
        ident = const.tile([128, 128], BF16, tag="ident")
        make_identity(nc, ident[:])
        iota_i = const.tile([128, K], I32, tag="iota_i")
        nc.gpsimd.iota(iota_i[:], pattern=[[1, K]], base=0, channel_multiplier=0)
        iota_f = const.tile([128, K], F32, tag="iota_f")
        nc.vector.tensor_copy(out=iota_f[:], in_=iota_i[:])

        # ---- accumulators ----
        es_ps = [p_esum.tile([128, K], F32, tag=f"esum{c}", name=f"esum{c}") for c in range(2)]
        cnt_ps = p_misc.tile([128, 8], F32, tag="cnt")
        nc.vector.memset(cnt_ps[:], 0.0)
        ssum_parts = acc.tile([128, N_TILES], F32, tag="ssum_parts")
        xsq_parts = acc.tile([128, 8], F32, tag="xsq_parts")
        sq_scratch = acc.tile([128, HW], F32, tag="sq_scratch")

        # per-batch bf16 splits
        xb_hi = [None, None]
        xb_mi = [None, None]

        prev = None  # deferred (onehot, xT_sbuf, tile_idx) for esum/counts

        for b in range(B_LOC):
            xb = [xpool.tile([128, HW], F32, tag=f"xb{c}", name=f"xb{c}") for c in range(2)]
            for c in range(2):
                nc.sync.dma_start(out=xb[c][:], in_=xloc[b, 128 * c:128 * (c + 1), :])
            resid = [xpool.tile([128, HW], F32, tag=f"res{c}", name=f"res{c}") for c in range(2)]
            xb_hi = [xpool.tile([128, HW], BF16, tag=f"xh{c}", name=f"xh{c}") for c in range(2)]
            xb_mi = [xpool.tile([128, HW], BF16, tag=f"xm{c}", name=f"xm{c}") for c in range(2)]
            for c in range(2):
                nc.scalar.activation(out=xb_hi[c][:], in_=xb[c][:],
                                     func=mybir.ActivationFunctionType.Copy)
                nc.vector.tensor_tensor(out=resid[c][:], in0=xb[c][:],
                                        in1=xb_hi[c][:], op=mybir.AluOpType.subtract)
                nc.scalar.activation(out=xb_mi[c][:], in_=resid[c][:],
                                     func=mybir.ActivationFunctionType.Copy)
                nc.vector.tensor_tensor(out=sq_scratch[:], in0=xb[c][:],
                                        in1=xb[c][:], op=mybir.AluOpType.mult)
                nc.vector.reduce_sum(out=xsq_parts[:, 2 * b + c:2 * b + c + 1],
                                     in_=sq_scratch[:], axis=mybir.AxisListType.X)

            for t in range(8):
                ti = b * 8 + t
                ts_ = slice(128 * t, 128 * (t + 1))

                # ---- distances (2 halves of K) ----
                dps = [p_dist.tile([128, 512], F32, tag="dist", name="dist") for _ in range(2)]
                for h in range(2):
                    hs = slice(512 * h, 512 * (h + 1))
                    mms = []
                    for c in range(2):
                        co = K * c
                        hs_c = slice(co + 512 * h, co + 512 * (h + 1))
                        mms += [(xb_hi[c][:, ts_], e_hi[:, hs_c]),
                                (xb_hi[c][:, ts_], e_mi[:, hs_c]),
                                (xb_mi[c][:, ts_], e_hi[:, hs_c])]
                    for j, (lhsT, rhs) in enumerate(mms):
                        nc.tensor.matmul(out=dps[h][:], lhsT=lhsT, rhs=rhs,
                                         start=(j == 0), stop=False)
                    nc.tensor.matmul(out=dps[h][:], lhsT=ones3[:], rhs=nesq[:, hs],
                                     start=False, stop=True)

                # ---- xT transposes (bf16) into one PSUM bank ----
                xt_ps = p_misc.tile([128, 512], BF16, tag="xt")
                for j, src in enumerate((xb_hi[0], xb_hi[1], xb_mi[0], xb_mi[1])):
                    nc.tensor.transpose(out=xt_ps[:, 128 * j:128 * (j + 1)],
                                        in_=src[:, ts_], identity=ident[:])
                xt_sb = work.tile([128, 512], BF16, tag="xt_sb")
                nc.scalar.activation(out=xt_sb[:], in_=xt_ps[:],
                                     func=mybir.ActivationFunctionType.Copy)

                # ---- argmax via top-8 per half, then combine ----
                mx = small.tile([128, 16], F32, tag="mx")
                mi = small.tile([128, 16], U32, tag="mi")
                for h in range(2):
                    nc.vector.max(out=mx[:, 8 * h:8 * h + 8], in_=dps[h][:])
                    nc.vector.max_index(out=mi[:, 8 * h:8 * h + 8],
                                        in_max=mx[:, 8 * h:8 * h + 8],
                                        in_values=dps[h][:])
                mif = small.tile([128, 2], F32, tag="mif")
                nc.vector.tensor_copy(out=mif[:], in_=mi[:, 0:16:8])
                # s* = max(vA, vB); code = vA>=vB ? iA : iB+512
                va, vb = mx[:, 0:1], mx[:, 8:9]
                nc.vector.tensor_tensor(out=ssum_parts[:, ti:ti + 1], in0=va,
                                        in1=vb, op=mybir.AluOpType.max)
                cmp = small.tile([128, 1], F32, tag="cmp")
                nc.vector.tensor_tensor(out=cmp[:], in0=va, in1=vb,
                                        op=mybir.AluOpType.is_ge)
                t1 = small.tile([128, 1], F32, tag="t1")
                nc.vector.tensor_scalar(out=t1[:], in0=mif[:, 1:2], scalar1=512.0,
                                        scalar2=None, op0=mybir.AluOpType.add)
                t2 = small.tile([128, 1], F32, tag="t2")
                nc.vector.tensor_tensor(out=t2[:], in0=mif[:, 0:1], in1=t1[:],
                                        op=mybir.AluOpType.subtract)
                nc.vector.tensor_tensor(out=t2[:], in0=cmp[:], in1=t2[:],
                                        op=mybir.AluOpType.mult)
                codes_f = small.tile([128, 1], F32, tag="codes_f")
                nc.vector.tensor_tensor(out=codes_f[:], in0=t1[:], in1=t2[:],
                                        op=mybir.AluOpType.add)
                codes_u = small.tile([128, 1], U32, tag="codes_u")
                nc.vector.tensor_copy(out=codes_u[:], in_=codes_f[:])
                nc.sync.dma_start(out=codes_out[128 * ti:128 * (ti + 1)],
                                  in_=codes_u[:, 0])

                # ---- one-hot (gpsimd) ----
                onehot = work.tile([128, K], BF16, tag="onehot")
                nc.vector.tensor_scalar(out=onehot[:], in0=iota_f[:],
                                        scalar1=codes_f[:], scalar2=None,
                                        op0=mybir.AluOpType.is_equal)

                # ---- gather x_q rows, store ----
                xq = small.tile([128, D], F32, tag="xq")
                nc.gpsimd.indirect_dma_start(
                    out=xq[:], out_offset=None, in_=emb_g[:, :],
                    in_offset=bass.IndirectOffsetOnAxis(ap=codes_u[:, :1], axis=0))
                nc.sync.dma_start(out=xq_out[128 * ti:128 * (ti + 1), :], in_=xq[:])

                # ---- deferred esum/counts from previous tile ----
                if prev is not None:
                    _esum_counts(nc, prev, es_ps, cnt_ps, ones_col, first=(prev[2] == 0))
                prev = (onehot, xt_sb, ti)

        _esum_counts(nc, prev, es_ps, cnt_ps, ones_col, first=False)

        # ================= tail =================
        cc_in = dram.tile([CC_LEN], F32, tag="cc_in")
        cc_out = dram.tile([CC_LEN], F32, tag="cc_out")
        es_sb = [work.tile([128, K], F32, tag=f"es_sb{c}", name=f"es_sb{c}") for c in range(2)]
        for c in range(2):
            nc.scalar.activation(out=es_sb[c][:], in_=es_ps[c][:],
                                 func=mybir.ActivationFunctionType.Copy)
            nc.sync.dma_start(
                out=cc_in[(CC_ESUM0, CC_ESUM1)[c]:(CC_ESUM0, CC_ESUM1)[c] + 128 * K],
                in_=es_sb[c][:])
        # counts [128p,8f] -> k-order flat
        cnt_sb = small.tile([128, 8], F32, tag="cnt_sb")
        nc.scalar.activation(out=cnt_sb[:], in_=cnt_ps[:],
                             func=mybir.ActivationFunctionType.Copy)
        nc.sync.dma_start(
            out=cc_in[CC_CNT:CC_CNT + K].rearrange("(c p) -> p c", p=128),
            in_=cnt_sb[:])
        xsq_v = small.tile([128, 1], F32, tag="xsq_v")
        ssum_v = small.tile([128, 1], F32, tag="ssum_v")
        nc.vector.reduce_sum(out=xsq_v[:], in_=xsq_parts[:], axis=mybir.AxisListType.X)
        nc.vector.reduce_sum(out=ssum_v[:], in_=ssum_parts[:], axis=mybir.AxisListType.X)
        scl = small.tile([128, 2], F32, tag="scl")
        nc.vector.tensor_copy(out=scl[:, 0:1], in_=xsq_v[:])
        nc.vector.tensor_copy(out=scl[:, 1:2], in_=ssum_v[:])
        nc.sync.dma_start(
            out=cc_in[CC_SCL:CC_SCL + 256].rearrange("(f p) -> p f", p=128),
            in_=scl[:])

        nc.gpsimd.collective_compute(
            "AllReduce", mybir.AluOpType.add,
            replica_groups=[list(range(N_CORES))],
            ins=[cc_in[:].opt()], outs=[cc_out[:].opt()])

        # ---- EMA math (all cores identically) ----
        esA = [work.tile([128, K], F32, tag=f"es_sb{c}", name=f"esA{c}") for c in range(2)]
        for c in range(2):
            nc.sync.dma_start(
                out=esA[c][:],
                in_=cc_out[(CC_ESUM0, CC_ESUM1)[c]:(CC_ESUM0, CC_ESUM1)[c] + 128 * K]
                .rearrange("(p k) -> p k", p=128))
        cnt_all = small.tile([128, 8], F32, tag="cnt_all")
        nc.sync.dma_start(
            out=cnt_all[:],
            in_=cc_out[CC_CNT:CC_CNT + K].rearrange("(c p) -> p c", p=128))
        scl_all = small.tile([128, 2], F32, tag="scl_all")
        nc.sync.dma_start(
            out=scl_all[:],
            in_=cc_out[CC_SCL:CC_SCL + 256].rearrange("(f p) -> p f", p=128))
        csz = small.tile([128, 8], F32, tag="csz")
        nc.sync.dma_start(out=csz[:], in_=cs_in[:].rearrange("(c p) -> p c", p=128))

        # ncs = DECAY*cs + (1-DECAY)*counts
        ncs = small.tile([128, 8], F32, tag="ncs")
        tA = small.tile([128, 8], F32, tag="tA")
        nc.vector.tensor_scalar(out=ncs[:], in0=csz[:], scalar1=DECAY,
                                scalar2=None, op0=mybir.AluOpType.mult)
        nc.vector.tensor_scalar(out=tA[:], in0=cnt_all[:], scalar1=1.0 - DECAY,
                                scalar2=None, op0=mybir.AluOpType.mult)
        nc.vector.tensor_tensor(out=ncs[:], in0=ncs[:], in1=tA[:],
                                op=mybir.AluOpType.add)
        nc.sync.dma_start(out=ncs_out[:].rearrange("(c p) -> p c", p=128), in_=ncs[:])

        # n = sum(ncs) -> broadcast [128,1]
        nrow = small.tile([128, 1], F32, tag="nrow")
        nc.vector.reduce_sum(out=nrow[:], in_=ncs[:], axis=mybir.AxisListType.X)
        n_ps = p_misc.tile([1, 1], F32, tag="xt")
        nc.tensor.matmul(out=n_ps[:], lhsT=nrow[:], rhs=ones_f1[:],
                         start=True, stop=True)
        n_sb = small.tile([1, 1], F32, tag="n_sb")
        nc.scalar.activation(out=n_sb[:], in_=n_ps[:],
                             func=mybir.ActivationFunctionType.Copy)
        nb_ps = p_misc.tile([128, 1], F32, tag="xt")
        nc.tensor.matmul(out=nb_ps[:], lhsT=ones_row[:], rhs=n_sb[:],
                         start=True, stop=True)
        nbc = small.tile([128, 1], F32, tag="nbc")
        nc.scalar.activation(out=nbc[:], in_=nb_ps[:],
                             func=mybir.ActivationFunctionType.Copy)

        # cs_k = (ncs+EPS) / (n + K*EPS) * n ; rec_cs = 1/cs_k
        denom = small.tile([128, 1], F32, tag="denom")
        nc.vector.tensor_scalar(out=denom[:], in0=nbc[:], scalar1=float(K) * EPS,
                                scalar2=None, op0=mybir.AluOpType.add)
        rden = small.tile([128, 1], F32, tag="rden")
        nc.vector.reciprocal(out=rden[:], in_=denom[:])
        fac = small.tile([128, 1], F32, tag="fac")
        nc.vector.tensor_tensor(out=fac[:], in0=nbc[:], in1=rden[:],
                                op=mybir.AluOpType.mult)
        cs_k = small.tile([128, 8], F32, tag="cs_k")
        nc.vector.tensor_scalar(out=cs_k[:], in0=ncs[:], scalar1=EPS,
                                scalar2=None, op0=mybir.AluOpType.add)
        nc.vector.tensor_scalar(out=cs_k[:], in0=cs_k[:], scalar1=fac[:],
                                scalar2=None, op0=mybir.AluOpType.mult)
        rcs = small.tile([128, 8], F32, tag="rcs")
        nc.vector.reciprocal(out=rcs[:], in_=cs_k[:])
        # roundtrip rec_cs to get row layout [1, 1024]
        rcs_d = dram.tile([K], F32, tag="rcs_d")
        nc.sync.dma_start(out=rcs_d[:].rearrange("(c p) -> p c", p=128), in_=rcs[:])
        rcs_row = small.tile([1, K], F32, tag="rcs_row")
        nc.sync.dma_start(out=rcs_row[:], in_=rcs_d[None, :])
        # broadcast to [128, 1024] via PE (2 halves)
        rb_ps = [p_dist.tile([128, 512], F32, tag="dist", name="rb_ps") for _ in range(2)]
        for h in range(2):
            nc.tensor.matmul(out=rb_ps[h][:], lhsT=ones_row[:],
                             rhs=rcs_row[:, 512 * h:512 * (h + 1)],
                             start=True, stop=True)

        # neaT = DECAY*avgT + (1-DECAY)*esum ; enormT = neaT * rec_cs
        avgT = [xpool.tile([128, K], F32, tag=f"xb{c}", name=f"avgT{c}") for c in range(2)]
        for c in range(2):
            nc.sync.dma_start(out=avgT[c][:], in_=avgT_in[128 * c:128 * (c + 1), :])
        for c in range(2):
            na = xpool.tile([128, K], F32, tag=f"res{c}")
            nc.scalar.activation(out=na[:], in_=esA[c][:],
                                 func=mybir.ActivationFunctionType.Copy,
                                 scale=1.0 - DECAY)
            nc.vector.tensor_scalar(out=avgT[c][:], in0=avgT[c][:], scalar1=DECAY,
                                    scalar2=None, op0=mybir.AluOpType.mult)
            nc.vector.tensor_tensor(out=na[:], in0=na[:], in1=avgT[c][:],
                                    op=mybir.AluOpType.add)
            nc.sync.dma_start(out=neaT_out[128 * c:128 * (c + 1), :], in_=na[:])
            en = xpool.tile([128, K], F32, tag=f"xh{c}")
            for h in range(2):
                nc.vector.tensor_tensor(out=en[:, 512 * h:512 * (h + 1)],
                                        in0=na[:, 512 * h:512 * (h + 1)],
                                        in1=rb_ps[h][:], op=mybir.AluOpType.mult)
            nc.sync.dma_start(out=enormT_out[128 * c:128 * (c + 1), :], in_=en[:])

        # loss = ((xsq - ssum) * LOSS_SCALE); xsq/ssum are partition-sums of scl_all
        sc_ps = p_misc.tile([2, 1], F32, tag="xt")
        nc.tensor.matmul(out=sc_ps[:], lhsT=scl_all[:], rhs=ones_f1[:],
                         start=True, stop=True)
        sc_sb = small.tile([2, 1], F32, tag="sc_sb")
        nc.scalar.activation(out=sc_sb[:], in_=sc_ps[:],
                             func=mybir.ActivationFunctionType.Copy)
        sc_d = dram.tile([2], F32, tag="sc_d")
        nc.sync.dma_start(out=sc_d[:], in_=sc_sb[:, 0])
        sc_row = small.tile([1, 2], F32, tag="sc_row")
        nc.sync.dma_start(out=sc_row[:], in_=sc_d[None, :])
        lt = small.tile([1, 1], F32, tag="lt")
        nc.vector.tensor_tensor(out=lt[:], in0=sc_row[:, 0:1], in1=sc_row[:, 1:2],
                                op=mybir.AluOpType.subtract)
        nc.vector.tensor_scalar(out=lt[:], in0=lt[:], scalar1=LOSS_SCALE,
                                scalar2=None, op0=mybir.AluOpType.mult)
        nc.sync.dma_start(out=loss_out[:], in_=lt[:, 0])


def _esum_counts(nc, prev, es_ps, cnt_ps, ones_col, first):
    onehot, xt_sb, ti = prev
    st = (ti == 0)
    for c in range(2):
        for s in range(2):  # hi, mi
            lhsT = xt_sb[:, 256 * s + 128 * c:256 * s + 128 * (c + 1)]
            for h in range(2):
                nc.tensor.matmul(out=es_ps[c][:, 512 * h:512 * (h + 1)],
                                 lhsT=lhsT, rhs=onehot[:, 512 * h:512 * (h + 1)],
                                 start=(st and s == 0), stop=False)
    for kc in range(8):
        nc.tensor.matmul(out=cnt_ps[:, kc:kc + 1],
                         lhsT=onehot[:, 128 * kc:128 * (kc + 1)], rhs=ones_col[:],
                         start=False, stop=False)


def _split3_neg_esq(emb):
    esq = np.sum(emb.astype(np.float64) * emb.astype(np.float64), axis=1)
    v = (-esq).astype(np.float32)
    h = v.astype(ml_dtypes.bfloat16)
    r = v - h.astype(np.float32)
    m = r.astype(ml_dtypes.bfloat16)
    l = (r - m.astype(np.float32)).astype(ml_dtypes.bfloat16)
    return np.stack([h, m, l])


def _split2(v):
    h = v.astype(ml_dtypes.bfloat16)
    m = (v - h.astype(np.float32)).astype(ml_dtypes.bfloat16)
    return h, m


def kernel(x, emb, cluster_size, embed_avg, _trace=False):
    x = np.ascontiguousarray(np.asarray(x, dtype=np.float32))
    emb = np.ascontiguousarray(np.asarray(emb, dtype=np.float32))
    cluster_size = np.asarray(cluster_size, dtype=np.float32)
    embed_avg = np.asarray(embed_avg, dtype=np.float32)

    if "nc" not in _CACHED:
        _CACHED["nc"] = build()
    nc = _CACHED["nc"]

    e2t = np.ascontiguousarray((2.0 * emb).T)            # [D, K]
    e2t_hi, e2t_mi = _split2(e2t)
    nesq3 = _split3_neg_esq(emb)                          # [3, K]
    avgT = np.ascontiguousarray(embed_avg.T)              # [D, K]

    in_maps = []
    for c in range(N_CORES):
        xs = x[B_LOC * c:B_LOC * (c + 1)].reshape(B_LOC, D, HW)
        in_maps.append({
            "xloc": np.ascontiguousarray(xs),
            "e2t_hi": e2t_hi, "e2t_mi": e2t_mi, "nesq3": nesq3,
            "emb_g": emb, "avgT_in": avgT, "cs_in": cluster_size,
        })

    res = run_bass_kernel_spmd(nc, in_maps, core_ids=list(range(N_CORES)),
                               trace=_trace)
    outs = res.results
    _CACHED["outs"] = outs
    if _trace:
        _CACHED["last_result"] = res

    # ---- unshard ----
    xq_st = np.empty((B, D, H, W), np.float32)
    codes_map = np.empty((B, HW), np.int32)
    for c in range(N_CORES):
        o = outs[c]
        xq = o["xq_out"].reshape(B_LOC, H, W, D).transpose(0, 3, 1, 2)
        xq_st[B_LOC * c:B_LOC * (c + 1)] = xq
        codes_map[B_LOC * c:B_LOC * (c + 1)] = \
            o["codes_out"].view(np.int32).reshape(B_LOC, HW)
    codes_map = codes_map.reshape(B, H, W)
    o0 = outs[0]
    vq_loss = np.float32(o0["loss_out"][0])
    new_cluster_size = o0["ncs_out"]
    new_embed_avg = np.ascontiguousarray(o0["neaT_out"].T)
    embed_normalized = np.ascontiguousarray(o0["enormT_out"].T)
    return (xq_st, vq_loss, codes_map, new_cluster_size, new_embed_avg,
            embed_normalized)
